# revision 1
# baseline (speedup 1.0000x reference)
"""Trainium2 Bass kernel for MySpikeGPT (spiking linear-attention transformer).

Strategy: data-parallel over the 8 (T, B) slices across 8 NeuronCores.
Activations are kept feature-major [D, CTX] on-chip, so every matmul
consumes weights in their natural [din, dout] layout (lhsT) and activations
as the moving operand; layernorm statistics (over features = partitions)
are computed with an all-ones [128,128] stationary matmul, which replicates
the column sums onto all 128 partitions (no cross-partition broadcast
needed). The spiking attention (elementwise Q*K summed over sequence) is a
free-dim reduction in this layout, done on the vector engine.

Launch 1: embedding-encoder + 12 transformer layers -> relu(h)/T partials.
Host: sums the 4 per-timestep partials of each batch element.
Launch 2: pooled @ w_out, vocab-sharded 4-way per batch element.
"""

import sys

if "/opt/trn_rl_repo" not in sys.path:
    sys.path.insert(0, "/opt/trn_rl_repo")

import numpy as np

import concourse.bacc as bacc
import concourse.tile as tile
from concourse import mybir
from concourse.bass_utils import run_bass_kernel_spmd

P = 128
V, D, CTX, HH, HD, FF, L, T, B = 50257, 768, 1024, 12, 64, 3072, 12, 4, 2
KD = D // P          # 6 feature tiles of h/q/k/v
KF = FF // P         # 24 feature tiles of f1
EPS = 1e-5
LAMB = 10000.0
NCORES = 8
NCH = 2              # token chunks per CTX in launch 1
NT = CTX // NCH      # 512 tokens per chunk
FFQ = 4              # w1/w2 streamed in quarters of 768 columns/rows
VPAD = 51200         # vocab padded to 4 shards x 25 N-chunks x 512
VSH = VPAD // 4      # 12800 vocab columns per core

F32 = mybir.dt.float32
F32R = mybir.dt.float32r
AF = mybir.ActivationFunctionType
ALU = mybir.AluOpType


def _poe() -> np.ndarray:
    i = np.arange(CTX, dtype=np.float32)[:, None]
    j = np.arange(D)
    expo = (j - (j % 2)).astype(np.float32) / D
    ang = i / (LAMB ** expo)
    return np.where(j % 2 == 0, np.sin(ang), np.cos(ang)).astype(np.float32)


def _ln_stats(nc, sps, work, ones, eps_t, sum_ps, sq_ps, inv_n, ns):
    """Turn accumulated sum/sumsq PSUM stats into m_full/rs_full (replicated
    on all partitions). Returns (m_full, rs_full)."""
    m_full = work.tile([P, NT], F32, tag="mfull")
    nc.scalar.activation(m_full, sum_ps[:], AF.Copy, scale=inv_n)
    e2 = work.tile([P, NT], F32, tag="e2")
    nc.scalar.activation(e2, sq_ps[:], AF.Copy, scale=inv_n)
    msq = work.tile([P, NT], F32, tag="msq")
    nc.vector.tensor_mul(msq, m_full, m_full)
    nc.vector.tensor_sub(e2, e2, msq)
    nc.scalar.activation(e2, e2, AF.Sqrt, bias=eps_t[:])
    rs_full = work.tile([P, NT], F32, tag="rsfull")
    nc.vector.reciprocal(rs_full, e2)
    return m_full, rs_full


def build_launch1(layers=L):
    nc = bacc.Bacc("TRN2", target_bir_lowering=False, debug=False,
                   num_devices=NCORES)

    x0_d = nc.dram_tensor("x0", [D, CTX], F32R, kind="ExternalInput")
    wq_d = nc.dram_tensor("wq", [layers, D, D], F32R, kind="ExternalInput")
    wk_d = nc.dram_tensor("wk", [layers, D, D], F32R, kind="ExternalInput")
    wv_d = nc.dram_tensor("wv", [layers, D, D], F32R, kind="ExternalInput")
    wo_d = nc.dram_tensor("wo", [layers, D, D], F32R, kind="ExternalInput")
    w1_d = nc.dram_tensor("w1", [layers, D, FF], F32R, kind="ExternalInput")
    w2_d = nc.dram_tensor("w2", [layers, FF, D], F32R, kind="ExternalInput")
    # w1 row-sums, pre-replicated across 128 columns (sum-stat shortcut)
    w1b_d = nc.dram_tensor("w1b", [layers, D, P], F32R, kind="ExternalInput")
    # per-layer D-sized LN params: qg qb kg kb vg vb og ob 2g 2b
    lnd_d = nc.dram_tensor("lnd", [layers, 10, D], F32, kind="ExternalInput")
    lnf_d = nc.dram_tensor("lnf", [layers, 2, FF], F32, kind="ExternalInput")
    enc_d = nc.dram_tensor("encp", [2, D], F32, kind="ExternalInput")
    hp_d = nc.dram_tensor("hpart", [D, CTX], F32, kind="ExternalOutput")

    with tile.TileContext(nc) as tc:
        with (
            tc.tile_pool(name="persist", bufs=1) as pers,
            tc.tile_pool(name="ybuf", bufs=1) as pyb,
            tc.tile_pool(name="wpool", bufs=2) as pw,
            tc.tile_pool(name="thalf", bufs=2) as pth,
            tc.tile_pool(name="tmp", bufs=3) as ptmp,
            tc.tile_pool(name="work", bufs=2) as pwork,
            tc.tile_pool(name="lnp", bufs=2) as plnp,
            tc.tile_pool(name="psmain", bufs=6, space="PSUM") as psp,
            tc.tile_pool(name="psstat", bufs=2, space="PSUM") as sps,
        ):
            ones32 = pers.tile([P, P], F32)
            nc.vector.memset(ones32, 1.0)
            ones = pers.tile([P, P], F32R)
            nc.vector.tensor_copy(ones, ones32)
            eps_t = pers.tile([P, 1], F32)
            nc.vector.memset(eps_t, EPS)
            h = pers.tile([P, KD, CTX], F32)
            q_t = pers.tile([P, KD, CTX], F32R, tag="qatt")
            qkraw = pers.tile([P, KD, NCH], F32)
            qk = pers.tile([P, KD], F32)
            bprime = pers.tile([P, KD], F32)

            # ---------------- encoder layernorm ----------------
            encp_t = plnp.tile([P, 2, KD], F32, tag="lnd")
            nc.sync.dma_start(encp_t[:], enc_d.rearrange("r (k p) -> p r k", p=P))
            x0 = pyb.tile([P, KD, CTX], F32R, tag="y")
            nc.sync.dma_start(x0[:], x0_d.rearrange("(k p) n -> p k n", p=P))
            for n in range(NCH):
                ns = slice(n * NT, (n + 1) * NT)
                sum_ps = sps.tile([P, NT], F32, tag="sp")
                sq_ps = sps.tile([P, NT], F32, tag="sp")
                for m in range(KD):
                    sq = ptmp.tile([P, NT], F32R, tag="tmp")
                    nc.vector.tensor_mul(sq, x0[:, m, ns], x0[:, m, ns])
                    nc.tensor.matmul(sum_ps[:], ones[:], x0[:, m, ns],
                                     start=(m == 0), stop=(m == KD - 1))
                    nc.tensor.matmul(sq_ps[:], ones[:], sq,
                                     start=(m == 0), stop=(m == KD - 1))
                m_full, rs_full = _ln_stats(nc, sps, pwork, ones, eps_t,
                                            sum_ps, sq_ps, 1.0 / D, ns)
                for m in range(KD):
                    nc.vector.tensor_sub(x0[:, m, ns], x0[:, m, ns], m_full)
                    nc.vector.scalar_tensor_tensor(
                        x0[:, m, ns], in0=x0[:, m, ns],
                        scalar=encp_t[:, 0, m:m + 1], in1=rs_full,
                        op0=ALU.mult, op1=ALU.mult)
                    nc.vector.tensor_scalar_add(
                        h[:, m, ns], x0[:, m, ns], encp_t[:, 1, m:m + 1])

            # ---------------- transformer layers ----------------
            for l in range(layers):
                lnd_t = plnp.tile([P, 10, KD], F32, tag="lnd")
                nc.sync.dma_start(lnd_t[:],
                                  lnd_d[l].rearrange("r (k p) -> p r k", p=P))
                lnf_t = plnp.tile([P, 2, KF], F32, tag="lnf")
                nc.sync.dma_start(lnf_t[:],
                                  lnf_d[l].rearrange("r (k p) -> p r k", p=P))

                # --- q, k, v, o projections ---
                for mode, w_src, grow, brow in (
                    ("q", wq_d, 0, 1), ("k", wk_d, 2, 3),
                    ("v", wv_d, 4, 5), ("o", wo_d, 6, 7),
                ):
                    w_t = pw.tile([P, KD, D], F32R, tag="w")
                    nc.sync.dma_start(
                        w_t[:], w_src[l].rearrange("(k p) m -> p k m", p=P))
                    y = pyb.tile([P, KD, CTX], F32R, tag="y")
                    for n in range(NCH):
                        ns = slice(n * NT, (n + 1) * NT)
                        if mode == "o":
                            rhs = [q_t[:, k, ns] for k in range(KD)]
                        else:
                            th = pth.tile([P, KD, NT], F32R, tag="th")
                            for k in range(KD):
                                nc.scalar.activation(th[:, k], h[:, k, ns],
                                                     AF.Relu)
                            rhs = [th[:, k] for k in range(KD)]
                        pss = []
                        for m in range(KD):
                            ps = psp.tile([P, NT], F32, tag="yp")
                            pss.append(ps)
                            for k in range(KD):
                                nc.tensor.matmul(
                                    ps[:], w_t[:, k, m * P:(m + 1) * P],
                                    rhs[k],
                                    start=(k == 0), stop=(k == KD - 1))
                        sum_ps = sps.tile([P, NT], F32, tag="sp")
                        sq_ps = sps.tile([P, NT], F32, tag="sp")
                        for m in range(KD):
                            nc.scalar.activation(y[:, m, ns], pss[m][:],
                                                 AF.Copy)
                            sq = ptmp.tile([P, NT], F32R, tag="tmp")
                            nc.vector.tensor_mul(sq, y[:, m, ns], y[:, m, ns])
                            nc.tensor.matmul(sum_ps[:], ones[:], y[:, m, ns],
                                             start=(m == 0), stop=(m == KD - 1))
                            nc.tensor.matmul(sq_ps[:], ones[:], sq,
                                             start=(m == 0), stop=(m == KD - 1))
                        m_full, rs_full = _ln_stats(nc, sps, pwork, ones,
                                                    eps_t, sum_ps, sq_ps,
                                                    1.0 / D, ns)
                        for m in range(KD):
                            nc.vector.tensor_sub(y[:, m, ns], y[:, m, ns],
                                                 m_full)
                            nc.vector.scalar_tensor_tensor(
                                y[:, m, ns], in0=y[:, m, ns],
                                scalar=lnd_t[:, grow, m:m + 1], in1=rs_full,
                                op0=ALU.mult, op1=ALU.mult)
                            if mode == "q":
                                nc.scalar.activation(
                                    q_t[:, m, ns], y[:, m, ns], AF.Relu,
                                    bias=lnd_t[:, brow, m:m + 1])
                            elif mode == "k":
                                kt = ptmp.tile([P, NT], F32R, tag="tmp")
                                nc.scalar.activation(
                                    kt, y[:, m, ns], AF.Relu,
                                    bias=lnd_t[:, brow, m:m + 1])
                                prod = ptmp.tile([P, NT], F32R, tag="tmp")
                                nc.vector.tensor_mul(prod, q_t[:, m, ns], kt)
                                nc.vector.tensor_reduce(
                                    qkraw[:, m, n:n + 1], prod,
                                    axis=mybir.AxisListType.X, op=ALU.add)
                            elif mode == "v":
                                # att = relu(w*qk + b*qk) into the q slot
                                nc.scalar.activation(
                                    q_t[:, m, ns], y[:, m, ns], AF.Relu,
                                    bias=bprime[:, m:m + 1],
                                    scale=qk[:, m:m + 1])
                            else:  # o: h += w + b
                                nc.vector.scalar_tensor_tensor(
                                    h[:, m, ns], in0=y[:, m, ns],
                                    scalar=lnd_t[:, brow, m:m + 1],
                                    in1=h[:, m, ns],
                                    op0=ALU.add, op1=ALU.add)
                    if mode == "k":
                        # qk = relu(sum_seq q*k); bprime = lnv_b * qk
                        nc.vector.tensor_add(qk[:, :], qkraw[:, :, 0],
                                             qkraw[:, :, 1])
                        nc.scalar.activation(qk[:, :], qk[:, :], AF.Relu)
                        nc.vector.tensor_mul(bprime[:, :], lnd_t[:, 5, :],
                                             qk[:, :])

                # --- FFN ---
                w1b_t = pw.tile([P, KD, P], F32R, tag="w1b")
                nc.sync.dma_start(
                    w1b_t[:], w1b_d[l].rearrange("(k p) c -> p k c", p=P))
                for n in range(NCH):
                    ns = slice(n * NT, (n + 1) * NT)
                    th = pth.tile([P, KD, NT], F32R, tag="th")
                    for k in range(KD):
                        nc.scalar.activation(th[:, k], h[:, k, ns], AF.Relu)
                    y1 = pyb.tile([P, KF, NT], F32R, tag="y")
                    sum_ps = sps.tile([P, NT], F32, tag="sp")
                    sq_ps = sps.tile([P, NT], F32, tag="sp")
                    # sum-stat via w1 row-sums
                    for k in range(KD):
                        nc.tensor.matmul(sum_ps[:], w1b_t[:, k], th[:, k],
                                         start=(k == 0), stop=(k == KD - 1))
                    for fq in range(FFQ):
                        w_t = pw.tile([P, KD, D], F32R, tag="w")
                        nc.sync.dma_start(
                            w_t[:],
                            w1_d[l][:, fq * D:(fq + 1) * D].rearrange(
                                "(k p) m -> p k m", p=P))
                        pss = []
                        for m in range(KD):
                            ps = psp.tile([P, NT], F32, tag="yp")
                            pss.append(ps)
                            for k in range(KD):
                                nc.tensor.matmul(
                                    ps[:], w_t[:, k, m * P:(m + 1) * P],
                                    th[:, k],
                                    start=(k == 0), stop=(k == KD - 1))
                        for m in range(KD):
                            mg = fq * KD + m
                            nc.scalar.activation(y1[:, mg], pss[m][:], AF.Copy)
                            sq = ptmp.tile([P, NT], F32R, tag="tmp")
                            nc.vector.tensor_mul(sq, y1[:, mg], y1[:, mg])
                            nc.tensor.matmul(sq_ps[:], ones[:], sq,
                                             start=(mg == 0),
                                             stop=(mg == KF - 1))
                    m_full, rs_full = _ln_stats(nc, sps, pwork, ones, eps_t,
                                                sum_ps, sq_ps, 1.0 / FF, ns)
                    for mg in range(KF):
                        nc.vector.tensor_sub(y1[:, mg], y1[:, mg], m_full)
                        nc.vector.scalar_tensor_tensor(
                            y1[:, mg], in0=y1[:, mg],
                            scalar=lnf_t[:, 0, mg:mg + 1], in1=rs_full,
                            op0=ALU.mult, op1=ALU.mult)
                        nc.scalar.activation(y1[:, mg], y1[:, mg], AF.Relu,
                                             bias=lnf_t[:, 1, mg:mg + 1])
                    # f2 = f1 @ w2, contracting all 24 feature tiles
                    pss2 = [psp.tile([P, NT], F32, tag="yp", name=f"ps2_{m}")
                            for m in range(KD)]
                    for qq in range(FFQ):
                        w_t = pw.tile([P, KD, D], F32R, tag="w")
                        nc.sync.dma_start(
                            w_t[:],
                            w2_d[l][qq * D:(qq + 1) * D].rearrange(
                                "(k p) m -> p k m", p=P))
                        for kk in range(KD):
                            for m in range(KD):
                                nc.tensor.matmul(
                                    pss2[m][:], w_t[:, kk, m * P:(m + 1) * P],
                                    y1[:, qq * KD + kk],
                                    start=(qq == 0 and kk == 0),
                                    stop=(qq == FFQ - 1 and kk == KD - 1))
                    yf = pyb.tile([P, KD, NT], F32R, tag="y")
                    sum_ps = sps.tile([P, NT], F32, tag="sp")
                    sq_ps = sps.tile([P, NT], F32, tag="sp")
                    for m in range(KD):
                        nc.scalar.activation(yf[:, m], pss2[m][:], AF.Copy)
                        sq = ptmp.tile([P, NT], F32R, tag="tmp")
                        nc.vector.tensor_mul(sq, yf[:, m], yf[:, m])
                        nc.tensor.matmul(sum_ps[:], ones[:], yf[:, m],
                                         start=(m == 0), stop=(m == KD - 1))
                        nc.tensor.matmul(sq_ps[:], ones[:], sq,
                                         start=(m == 0), stop=(m == KD - 1))
                    m_full, rs_full = _ln_stats(nc, sps, pwork, ones, eps_t,
                                                sum_ps, sq_ps, 1.0 / D, ns)
                    for m in range(KD):
                        nc.vector.tensor_sub(yf[:, m], yf[:, m], m_full)
                        nc.vector.scalar_tensor_tensor(
                            yf[:, m], in0=yf[:, m],
                            scalar=lnd_t[:, 8, m:m + 1], in1=rs_full,
                            op0=ALU.mult, op1=ALU.mult)
                        nc.vector.scalar_tensor_tensor(
                            h[:, m, ns], in0=yf[:, m],
                            scalar=lnd_t[:, 9, m:m + 1], in1=h[:, m, ns],
                            op0=ALU.add, op1=ALU.add)

            # ---------------- pooled partial ----------------
            hp = pyb.tile([P, KD, CTX], F32, tag="y")
            for m in range(KD):
                nc.scalar.activation(hp[:, m], h[:, m], AF.Relu,
                                     scale=1.0 / T)
            nc.sync.dma_start(hp_d.rearrange("(k p) n -> p k n", p=P), hp[:])

    nc.compile()
    return nc


def build_launch2():
    nc = bacc.Bacc("TRN2", target_bir_lowering=False, debug=False,
                   num_devices=NCORES)
    pooled_d = nc.dram_tensor("pooled", [D, CTX], F32R, kind="ExternalInput")
    wsh_d = nc.dram_tensor("wsh", [D, VSH], F32R, kind="ExternalInput")
    out_d = nc.dram_tensor("logits", [CTX, VSH], F32, kind="ExternalOutput")

    NV = VSH // 512  # 25 vocab chunks per core

    with tile.TileContext(nc) as tc:
        with (
            tc.tile_pool(name="pld", bufs=1) as pld,
            tc.tile_pool(name="wp", bufs=3) as pw,
            tc.tile_pool(name="op", bufs=4) as po,
            tc.tile_pool(name="ps", bufs=8, space="PSUM") as psp,
        ):
            pt = pld.tile([P, KD, CTX], F32R)
            nc.sync.dma_start(pt[:], pooled_d.rearrange("(k p) n -> p k n",
                                                        p=P))
            for v in range(NV):
                w_t = pw.tile([P, KD, 512], F32R, tag="w")
                nc.sync.dma_start(
                    w_t[:],
                    wsh_d[:, v * 512:(v + 1) * 512].rearrange(
                        "(k p) n -> p k n", p=P))
                for m in range(CTX // P):
                    ps = psp.tile([P, 512], F32, tag="ps")
                    for k in range(KD):
                        nc.tensor.matmul(ps[:], pt[:, k, m * P:(m + 1) * P],
                                         w_t[:, k],
                                         start=(k == 0), stop=(k == KD - 1))
                    ot = po.tile([P, 512], F32, tag="o")
                    nc.scalar.activation(ot, ps[:], AF.Copy)
                    nc.sync.dma_start(
                        out_d[m * P:(m + 1) * P, v * 512:(v + 1) * 512], ot)
    nc.compile()
    return nc


_CACHE = {}


def _get_launch1(layers=L):
    key = ("l1", layers)
    if key not in _CACHE:
        _CACHE[key] = build_launch1(layers)
    return _CACHE[key]


def _get_launch2():
    if "l2" not in _CACHE:
        _CACHE["l2"] = build_launch2()
    return _CACHE["l2"]


def kernel(tokens, emb, enc_g, enc_b, wq, wk, wv, wo,
           lnq_g, lnq_b, lnk_g, lnk_b, lnv_g, lnv_b, lno_g, lno_b,
           w1, ln1_g, ln1_b, w2, ln2_g, ln2_b, w_out,
           _layers=L, _trace=False):
    f32 = np.float32
    tokens = np.asarray(tokens)
    poe = _poe()

    # per-core (t, b) slices of the encoded input, feature-major
    x0s = []
    for c in range(NCORES):
        b, t = divmod(c, T)
        x = np.asarray(emb, f32)[tokens[t, b]] + poe       # [CTX, D]
        x0s.append(np.ascontiguousarray(x.T, f32))         # [D, CTX]

    lnd = np.ascontiguousarray(
        np.stack([lnq_g, lnq_b, lnk_g, lnk_b, lnv_g, lnv_b,
                  lno_g, lno_b, ln2_g, ln2_b], axis=1), f32)  # [L,10,D]
    lnf = np.ascontiguousarray(
        np.stack([ln1_g, ln1_b], axis=1), f32)                # [L,2,FF]
    encp = np.stack([enc_g, enc_b]).astype(f32)               # [2,D]
    w1b = np.ascontiguousarray(
        np.repeat(np.asarray(w1, f32).sum(axis=2)[:, :, None], P, axis=2))

    shared = {
        "wq": np.ascontiguousarray(wq, f32),
        "wk": np.ascontiguousarray(wk, f32),
        "wv": np.ascontiguousarray(wv, f32),
        "wo": np.ascontiguousarray(wo, f32),
        "w1": np.ascontiguousarray(w1, f32),
        "w2": np.ascontiguousarray(w2, f32),
        "w1b": w1b, "lnd": lnd, "lnf": lnf, "encp": encp,
    }
    if _layers != L:
        for k in ("wq", "wk", "wv", "wo", "w1", "w2", "w1b", "lnd", "lnf"):
            shared[k] = np.ascontiguousarray(shared[k][:_layers])

    nc1 = _get_launch1(_layers)
    in_maps = [{"x0": x0s[c], **shared} for c in range(NCORES)]
    res1 = run_bass_kernel_spmd(nc1, in_maps, core_ids=list(range(NCORES)),
                                trace=_trace)
    hparts = [res1.results[c]["hpart"] for c in range(NCORES)]

    pooled = [np.sum([hparts[b * T + t] for t in range(T)], axis=0,
                     dtype=f32) for b in range(B)]

    w_out_pad = np.zeros((D, VPAD), f32)
    w_out_pad[:, :V] = np.asarray(w_out, f32)

    nc2 = _get_launch2()
    in_maps2 = []
    for c in range(NCORES):
        b, s = divmod(c, 4)
        in_maps2.append({
            "pooled": pooled[b],
            "wsh": np.ascontiguousarray(w_out_pad[:, s * VSH:(s + 1) * VSH]),
        })
    res2 = run_bass_kernel_spmd(nc2, in_maps2, core_ids=list(range(NCORES)),
                                trace=_trace)

    out = np.empty((B, CTX, V), f32)
    for b in range(B):
        full = np.concatenate(
            [res2.results[b * 4 + s]["logits"] for s in range(4)], axis=1)
        out[b] = full[:, :V]

    exec_ns = []
    for r in (res1, res2):
        if r.exec_time_ns is not None:
            exec_ns.append(r.exec_time_ns)
    if _trace and exec_ns:
        kernel.last_exec_ns = exec_ns
        kernel.last_results = (res1, res2)
    return out



# revision 10
# speedup vs baseline: 1.4376x; 1.4376x over previous
"""Trainium2 Bass kernel for MySpikeGPT (spiking linear-attention transformer).

Strategy: data-parallel over the 8 (T, B) slices across 8 NeuronCores.
Activations are feature-major [D, CTX] on-chip; layernorm statistics (over
features = partitions) are computed with an all-ones [128,128] stationary
matmul that replicates column sums onto all partitions. The spiking
attention (elementwise Q*K summed over sequence) is a fused
tensor_tensor_reduce on the vector engine.

v2 changes vs v1:
- fp16 weights + activations (fp32 PSUM accumulation): 2x DVE throughput,
  half the weight DMA, cheaper LDWEIGHTS. rel_err ~2.6e-3 (sim).
- LN scale via reciprocal_approx_fast (1 DVE op) instead of the 3.3us
  exact reciprocal; mean/var extraction moved to the scalar engine.
- relu(h) computed once per chunk and shared by the q/k/v and FFN GEMMs.
- V-apply folds the qk spike gate into the LN gamma/beta (qk >= 0 so
  relu(qk*(z+b)) == qk*relu(z+b)), decoupling the O-projection GEMM from
  the QK reduction.

Launch 1: embedding-encoder + 12 transformer layers -> relu(h)/T partials.
Host: sums the 4 per-timestep partials of each batch element.
Launch 2: pooled @ w_out, vocab-sharded 4-way per batch element.
"""

import sys

if "/opt/trn_rl_repo" not in sys.path:
    sys.path.insert(0, "/opt/trn_rl_repo")

import numpy as np

import concourse.bacc as bacc
import concourse.tile as tile
from concourse import mybir
from concourse.bass_utils import run_bass_kernel_spmd

P = 128
V, D, CTX, HH, HD, FF, L, T, B = 50257, 768, 1024, 12, 64, 3072, 12, 4, 2
KD = D // P          # 6 feature tiles of h/q/k/v
KF = FF // P         # 24 feature tiles of f1
EPS = 1e-5
LAMB = 10000.0
NCORES = 8
NCH = 2              # token chunks per CTX in launch 1
NT = CTX // NCH      # 512 tokens per chunk
FFQ = 4              # w1/w2 streamed in quarters of 768 columns/rows
VPAD = 51200         # vocab padded to 4 shards x 25 N-chunks x 512
VSH = VPAD // 4      # 12800 vocab columns per core

SQSC = 64.0          # sq tiles hold y^2/SQSC so fp16 never overflows
F32 = mybir.dt.float32
F16 = mybir.dt.float16
AF = mybir.ActivationFunctionType
ALU = mybir.AluOpType


def _poe() -> np.ndarray:
    i = np.arange(CTX, dtype=np.float32)[:, None]
    j = np.arange(D)
    expo = (j - (j % 2)).astype(np.float32) / D
    ang = i / (LAMB ** expo)
    return np.where(j % 2 == 0, np.sin(ang), np.cos(ang)).astype(np.float32)


def build_launch1(layers=L):
    nc = bacc.Bacc("TRN2", target_bir_lowering=False, debug=False,
                   num_devices=NCORES)

    x0_d = nc.dram_tensor("x0", [D, CTX], F16, kind="ExternalInput")
    wq_d = nc.dram_tensor("wq", [layers, D, D], F16, kind="ExternalInput")
    wk_d = nc.dram_tensor("wk", [layers, D, D], F16, kind="ExternalInput")
    wv_d = nc.dram_tensor("wv", [layers, D, D], F16, kind="ExternalInput")
    wo_d = nc.dram_tensor("wo", [layers, D, D], F16, kind="ExternalInput")
    w1_d = nc.dram_tensor("w1", [layers, D, FF], F16, kind="ExternalInput")
    w2_d = nc.dram_tensor("w2", [layers, FF, D], F16, kind="ExternalInput")
    # w1 row-sums, pre-replicated across 128 columns (sum-stat shortcut)
    w1b_d = nc.dram_tensor("w1b", [layers, D, P], F16, kind="ExternalInput")
    # per-layer D-sized LN params: qg qb kg kb vg vb og ob 2g 2b
    lnd_d = nc.dram_tensor("lnd", [layers, 10, D], F32, kind="ExternalInput")
    lnf_d = nc.dram_tensor("lnf", [layers, 2, FF], F32, kind="ExternalInput")
    enc_d = nc.dram_tensor("encp", [2, D], F32, kind="ExternalInput")
    hp_d = nc.dram_tensor("hpart", [D, CTX], F16, kind="ExternalOutput")

    with tile.TileContext(nc) as tc:
        with (
            tc.tile_pool(name="persist", bufs=1) as pers,
            tc.tile_pool(name="ybuf", bufs=3) as pyb,
            tc.tile_pool(name="y1buf", bufs=2) as py1,
            tc.tile_pool(name="sqbuf", bufs=2) as psq,
            tc.tile_pool(name="wpool", bufs=2) as pw,
            tc.tile_pool(name="w1bpool", bufs=2) as pwb,
            tc.tile_pool(name="stat16", bufs=4) as pst,
            tc.tile_pool(name="tmp32", bufs=4) as pt32,
            tc.tile_pool(name="ktmp", bufs=3) as pkt,
            tc.tile_pool(name="lnp", bufs=2) as plnp,
            tc.tile_pool(name="psmain", bufs=6, space="PSUM") as psp,
            tc.tile_pool(name="psstat", bufs=2, space="PSUM") as sps,
        ):
            ones32 = pers.tile([P, P], F32)
            nc.vector.memset(ones32, 1.0)
            ones = pers.tile([P, P], F16)
            nc.vector.tensor_copy(ones, ones32)
            h = pers.tile([P, KD, CTX], F32)
            x0 = pers.tile([P, KD, CTX], F16)       # reused as hp at the end
            qa = pers.tile([P, KD, CTX], F16)       # Q, then att (o-proj rhs)
            th = pers.tile([P, NCH, KD, NT], F16)   # relu(h): qkv, then ffn
            qkraw = pers.tile([P, KD, NCH], F32)
            qk32 = pers.tile([P, KD], F32)
            qkr = pers.tile([P, KD], F32)
            gv = pers.tile([P, KD], F32)
            bpr = pers.tile([P, KD], F32)

            def stats(sum_ps, sq_ps, inv_n):
                """PSUM sum/sumsq -> (mean_f16, rsqrt(var)_f16), replicated.
                sq_ps holds sum(y^2)/SQSC (scaled to dodge fp16 overflow)."""
                m16 = pst.tile([P, NT], F16, tag="m16")
                nc.scalar.activation(m16, sum_ps[:], AF.Copy, scale=inv_n)
                msq = pt32.tile([P, NT], F32, tag="t32")
                nc.scalar.activation(msq, sum_ps[:], AF.Square, scale=inv_n)
                e2 = pt32.tile([P, NT], F32, tag="t32")
                nc.scalar.activation(e2, sq_ps[:], AF.Copy,
                                     scale=inv_n * SQSC, bias=EPS)
                var = pt32.tile([P, NT], F32, tag="t32")
                nc.vector.tensor_sub(var, e2, msq)
                rinv = pt32.tile([P, NT], F32, tag="t32")
                nc.vector.reciprocal_approx_fast(out=rinv, in_=var)
                rs16 = pst.tile([P, NT], F16, tag="rs16")
                nc.scalar.activation(rs16, rinv, AF.Sqrt)
                return m16, rs16

            def copies_and_stats(pss, y, nm, inv_n, sq_extra=None):
                """PSUM tiles -> fp16 y[:, m]; emit sum+sq stat matmuls.
                Returns (m16, rs16). sq_extra: (sq_ps, start, stop) to
                accumulate sq into an existing PSUM instead of a new pair."""
                nmt = len(pss)
                sqt = psq.tile([P, nmt, NT], F16, tag="sq")
                for m in range(nmt):
                    nc.scalar.activation(y[:, m], pss[m][:], AF.Copy)
                    nc.vector.scalar_tensor_tensor(
                        sqt[:, m], in0=y[:, m], scalar=1.0 / SQSC,
                        in1=y[:, m], op0=ALU.mult, op1=ALU.mult)
                sum_ps = sps.tile([P, NT], F32, tag="sp")
                sq_ps = sps.tile([P, NT], F32, tag="sp")
                for m in range(nmt):
                    nc.tensor.matmul(sum_ps[:], ones[:], y[:, m],
                                     start=(m == 0), stop=(m == nmt - 1))
                for m in range(nmt):
                    nc.tensor.matmul(sq_ps[:], ones[:], sqt[:, m],
                                     start=(m == 0), stop=(m == nmt - 1))
                return stats(sum_ps, sq_ps, inv_n)

            # ---------------- encoder layernorm ----------------
            encp_t = plnp.tile([P, 2, KD], F32, tag="lnd")
            nc.sync.dma_start(encp_t[:], enc_d.rearrange("r (k p) -> p r k", p=P))
            nc.sync.dma_start(x0[:], x0_d.rearrange("(k p) n -> p k n", p=P))
            for n in range(NCH):
                ns = slice(n * NT, (n + 1) * NT)
                sqt = psq.tile([P, KD, NT], F16, tag="sq")
                sum_ps = sps.tile([P, NT], F32, tag="sp")
                sq_ps = sps.tile([P, NT], F32, tag="sp")
                for m in range(KD):
                    nc.vector.scalar_tensor_tensor(
                        sqt[:, m], in0=x0[:, m, ns], scalar=1.0 / SQSC,
                        in1=x0[:, m, ns], op0=ALU.mult, op1=ALU.mult)
                for m in range(KD):
                    nc.tensor.matmul(sum_ps[:], ones[:], x0[:, m, ns],
                                     start=(m == 0), stop=(m == KD - 1))
                for m in range(KD):
                    nc.tensor.matmul(sq_ps[:], ones[:], sqt[:, m],
                                     start=(m == 0), stop=(m == KD - 1))
                m16, rs16 = stats(sum_ps, sq_ps, 1.0 / D)
                for m in range(KD):
                    nc.vector.tensor_sub(x0[:, m, ns], x0[:, m, ns], m16)
                    nc.vector.scalar_tensor_tensor(
                        x0[:, m, ns], in0=x0[:, m, ns],
                        scalar=encp_t[:, 0, m:m + 1], in1=rs16,
                        op0=ALU.mult, op1=ALU.mult)
                    nc.vector.tensor_scalar_add(
                        h[:, m, ns], x0[:, m, ns], encp_t[:, 1, m:m + 1])

            # ---------------- transformer layers ----------------
            for l in range(layers):
                lnd_t = plnp.tile([P, 10, KD], F32, tag="lnd")
                nc.sync.dma_start(lnd_t[:],
                                  lnd_d[l].rearrange("r (k p) -> p r k", p=P))
                lnf_t = plnp.tile([P, 2, KF], F32, tag="lnf")
                nc.sync.dma_start(lnf_t[:],
                                  lnf_d[l].rearrange("r (k p) -> p r k", p=P))

                # shared relu(h) for q/k/v
                for n in range(NCH):
                    ns = slice(n * NT, (n + 1) * NT)
                    for m in range(KD):
                        nc.scalar.activation(th[:, n, m], h[:, m, ns], AF.Relu)

                # --- q, k, v, o projections ---
                for mode, w_src, grow, brow in (
                    ("q", wq_d, 0, 1), ("k", wk_d, 2, 3),
                    ("v", wv_d, 4, 5), ("o", wo_d, 6, 7),
                ):
                    w_t = pw.tile([P, KD, D], F16, tag="w")
                    nc.sync.dma_start(
                        w_t[:], w_src[l].rearrange("(k p) m -> p k m", p=P))
                    for n in range(NCH):
                        ns = slice(n * NT, (n + 1) * NT)
                        if mode == "o":
                            rhs = [qa[:, k, ns] for k in range(KD)]
                        else:
                            rhs = [th[:, n, k] for k in range(KD)]
                        pss = []
                        for m in range(KD):
                            ps = psp.tile([P, NT], F32, tag="yp")
                            pss.append(ps)
                            for k in range(KD):
                                nc.tensor.matmul(
                                    ps[:], w_t[:, k, m * P:(m + 1) * P],
                                    rhs[k],
                                    start=(k == 0), stop=(k == KD - 1))
                        y = pyb.tile([P, KD, NT], F16, tag="y")
                        m16, rs16 = copies_and_stats(pss, y, NT, 1.0 / D)
                        for m in range(KD):
                            nc.vector.tensor_sub(y[:, m], y[:, m], m16)
                            if mode == "v":
                                # fold qk gate: relu(qk*(z+b)) = qk*relu(z+b)
                                nc.vector.scalar_tensor_tensor(
                                    y[:, m], in0=y[:, m],
                                    scalar=gv[:, m:m + 1], in1=rs16,
                                    op0=ALU.mult, op1=ALU.mult)
                                nc.scalar.activation(
                                    qa[:, m, ns], y[:, m], AF.Relu,
                                    bias=bpr[:, m:m + 1])
                            elif mode == "o":
                                nc.vector.scalar_tensor_tensor(
                                    y[:, m], in0=y[:, m],
                                    scalar=lnd_t[:, grow, m:m + 1], in1=rs16,
                                    op0=ALU.mult, op1=ALU.mult)
                                # h += y + ob
                                nc.vector.scalar_tensor_tensor(
                                    h[:, m, ns], in0=y[:, m],
                                    scalar=lnd_t[:, brow, m:m + 1],
                                    in1=h[:, m, ns],
                                    op0=ALU.add, op1=ALU.add)
                            else:
                                nc.vector.scalar_tensor_tensor(
                                    y[:, m], in0=y[:, m],
                                    scalar=lnd_t[:, grow, m:m + 1], in1=rs16,
                                    op0=ALU.mult, op1=ALU.mult)
                                if mode == "q":
                                    nc.scalar.activation(
                                        qa[:, m, ns], y[:, m], AF.Relu,
                                        bias=lnd_t[:, brow, m:m + 1])
                                else:  # k: relu, then fused QK reduce
                                    kt = pkt.tile([P, NT], F16, tag="kt")
                                    nc.scalar.activation(
                                        kt, y[:, m], AF.Relu,
                                        bias=lnd_t[:, brow, m:m + 1])
                                    prod = pkt.tile([P, NT], F16, tag="prod")
                                    nc.vector.tensor_mul(prod, qa[:, m, ns],
                                                         kt)
                                    nc.vector.tensor_reduce(
                                        qkraw[:, m, n:n + 1], prod,
                                        axis=mybir.AxisListType.X, op=ALU.add)
                    if mode == "k":
                        # qk = relu(sum_seq q*k); fold into v's LN params
                        nc.vector.tensor_add(qk32[:, :], qkraw[:, :, 0],
                                             qkraw[:, :, 1])
                        nc.scalar.activation(qkr[:, :], qk32[:, :], AF.Relu)
                        nc.vector.tensor_mul(gv[:, :], lnd_t[:, 4, :],
                                             qkr[:, :])
                        nc.vector.tensor_mul(bpr[:, :], lnd_t[:, 5, :],
                                             qkr[:, :])

                # --- FFN ---
                w1b_t = pwb.tile([P, KD, P], F16, tag="w1b")
                nc.sync.dma_start(
                    w1b_t[:], w1b_d[l].rearrange("(k p) c -> p k c", p=P))
                # relu(h) after attention update (reuses th buffer)
                for n in range(NCH):
                    ns = slice(n * NT, (n + 1) * NT)
                    for m in range(KD):
                        nc.scalar.activation(th[:, n, m], h[:, m, ns], AF.Relu)
                y1s = []
                for n in range(NCH):
                    sum_ps = sps.tile([P, NT], F32, tag="sp")
                    sq_ps = sps.tile([P, NT], F32, tag="sp")
                    # sum-stat via w1 row-sums
                    for k in range(KD):
                        nc.tensor.matmul(sum_ps[:], w1b_t[:, k], th[:, n, k],
                                         start=(k == 0), stop=(k == KD - 1))
                    y1 = py1.tile([P, KF, NT], F16, tag="y1")
                    y1s.append(y1)
                    for fq in range(FFQ):
                        w_t = pw.tile([P, KD, D], F16, tag="w")
                        nc.sync.dma_start(
                            w_t[:],
                            w1_d[l][:, fq * D:(fq + 1) * D].rearrange(
                                "(k p) m -> p k m", p=P))
                        pss = []
                        for m in range(KD):
                            ps = psp.tile([P, NT], F32, tag="yp")
                            pss.append(ps)
                            for k in range(KD):
                                nc.tensor.matmul(
                                    ps[:], w_t[:, k, m * P:(m + 1) * P],
                                    th[:, n, k],
                                    start=(k == 0), stop=(k == KD - 1))
                        sqt = psq.tile([P, KD, NT], F16, tag="sq")
                        for m in range(KD):
                            mg = fq * KD + m
                            nc.scalar.activation(y1[:, mg], pss[m][:], AF.Copy)
                            nc.vector.scalar_tensor_tensor(
                                sqt[:, m], in0=y1[:, mg], scalar=1.0 / SQSC,
                                in1=y1[:, mg], op0=ALU.mult, op1=ALU.mult)
                        for m in range(KD):
                            mg = fq * KD + m
                            nc.tensor.matmul(sq_ps[:], ones[:], sqt[:, m],
                                             start=(mg == 0),
                                             stop=(mg == KF - 1))
                    m16, rs16 = stats(sum_ps, sq_ps, 1.0 / FF)
                    for mg in range(KF):
                        nc.vector.tensor_sub(y1[:, mg], y1[:, mg], m16)
                        nc.vector.scalar_tensor_tensor(
                            y1[:, mg], in0=y1[:, mg],
                            scalar=lnf_t[:, 0, mg:mg + 1], in1=rs16,
                            op0=ALU.mult, op1=ALU.mult)
                        nc.scalar.activation(y1[:, mg], y1[:, mg], AF.Relu,
                                             bias=lnf_t[:, 1, mg:mg + 1])
                # f2 = f1 @ w2, contracting all 24 feature tiles
                for n in range(NCH):
                    ns = slice(n * NT, (n + 1) * NT)
                    y1 = y1s[n]
                    pss2 = [psp.tile([P, NT], F32, tag="yp", name=f"ps2_{m}")
                            for m in range(KD)]
                    for qq in range(FFQ):
                        w_t = pw.tile([P, KD, D], F16, tag="w")
                        nc.sync.dma_start(
                            w_t[:],
                            w2_d[l][qq * D:(qq + 1) * D].rearrange(
                                "(k p) m -> p k m", p=P))
                        for kk in range(KD):
                            for m in range(KD):
                                nc.tensor.matmul(
                                    pss2[m][:], w_t[:, kk, m * P:(m + 1) * P],
                                    y1[:, qq * KD + kk],
                                    start=(qq == 0 and kk == 0),
                                    stop=(qq == FFQ - 1 and kk == KD - 1))
                    yf = pyb.tile([P, KD, NT], F16, tag="y")
                    m16, rs16 = copies_and_stats(pss2, yf, NT, 1.0 / D)
                    for m in range(KD):
                        nc.vector.tensor_sub(yf[:, m], yf[:, m], m16)
                        nc.vector.scalar_tensor_tensor(
                            yf[:, m], in0=yf[:, m],
                            scalar=lnd_t[:, 8, m:m + 1], in1=rs16,
                            op0=ALU.mult, op1=ALU.mult)
                        nc.vector.scalar_tensor_tensor(
                            h[:, m, ns], in0=yf[:, m],
                            scalar=lnd_t[:, 9, m:m + 1], in1=h[:, m, ns],
                            op0=ALU.add, op1=ALU.add)

            # ---------------- pooled partial ----------------
            for m in range(KD):
                nc.scalar.activation(x0[:, m], h[:, m], AF.Relu,
                                     scale=1.0 / T)
            nc.sync.dma_start(hp_d.rearrange("(k p) n -> p k n", p=P), x0[:])

    nc.compile()
    return nc


def build_launch2():
    nc = bacc.Bacc("TRN2", target_bir_lowering=False, debug=False,
                   num_devices=NCORES)
    pooled_d = nc.dram_tensor("pooled", [D, CTX], F16, kind="ExternalInput")
    wsh_d = nc.dram_tensor("wsh", [D, VSH], F16, kind="ExternalInput")
    out_d = nc.dram_tensor("logits", [CTX, VSH], F16, kind="ExternalOutput")

    NV = VSH // 512  # 25 vocab chunks per core

    with tile.TileContext(nc) as tc:
        with (
            tc.tile_pool(name="pld", bufs=1) as pld,
            tc.tile_pool(name="wp", bufs=3) as pw,
            tc.tile_pool(name="op", bufs=4) as po,
            tc.tile_pool(name="ps", bufs=8, space="PSUM") as psp,
        ):
            pt = pld.tile([P, KD, CTX], F16)
            nc.sync.dma_start(pt[:], pooled_d.rearrange("(k p) n -> p k n",
                                                        p=P))
            for v in range(NV):
                w_t = pw.tile([P, KD, 512], F16, tag="w")
                nc.sync.dma_start(
                    w_t[:],
                    wsh_d[:, v * 512:(v + 1) * 512].rearrange(
                        "(k p) n -> p k n", p=P))
                for m in range(CTX // P):
                    ps = psp.tile([P, 512], F32, tag="ps")
                    for k in range(KD):
                        nc.tensor.matmul(ps[:], pt[:, k, m * P:(m + 1) * P],
                                         w_t[:, k],
                                         start=(k == 0), stop=(k == KD - 1))
                    ot = po.tile([P, 512], F16, tag="o")
                    nc.scalar.activation(ot, ps[:], AF.Copy)
                    nc.sync.dma_start(
                        out_d[m * P:(m + 1) * P, v * 512:(v + 1) * 512], ot)
    nc.compile()
    return nc


_CACHE = {}


def _get_launch1(layers=L):
    key = ("l1", layers)
    if key not in _CACHE:
        _CACHE[key] = build_launch1(layers)
    return _CACHE[key]


def _get_launch2():
    if "l2" not in _CACHE:
        _CACHE["l2"] = build_launch2()
    return _CACHE["l2"]


def kernel(tokens, emb, enc_g, enc_b, wq, wk, wv, wo,
           lnq_g, lnq_b, lnk_g, lnk_b, lnv_g, lnv_b, lno_g, lno_b,
           w1, ln1_g, ln1_b, w2, ln2_g, ln2_b, w_out,
           _layers=L, _trace=False):
    f32, f16 = np.float32, np.float16
    tokens = np.asarray(tokens)
    poe = _poe()

    # per-core (t, b) slices of the encoded input, feature-major
    x0s = []
    for c in range(NCORES):
        b, t = divmod(c, T)
        x = np.asarray(emb, f32)[tokens[t, b]] + poe       # [CTX, D]
        x0s.append(np.ascontiguousarray(x.T).astype(f16))  # [D, CTX]

    lnd = np.ascontiguousarray(
        np.stack([lnq_g, lnq_b, lnk_g, lnk_b, lnv_g, lnv_b,
                  lno_g, lno_b, ln2_g, ln2_b], axis=1), f32)  # [L,10,D]
    lnf = np.ascontiguousarray(
        np.stack([ln1_g, ln1_b], axis=1), f32)                # [L,2,FF]
    encp = np.stack([enc_g, enc_b]).astype(f32)               # [2,D]

    w1_16 = np.ascontiguousarray(np.asarray(w1, f32)).astype(f16)
    w1b = np.repeat(
        w1_16.astype(f32).sum(axis=2, dtype=f32)[:, :, None], P, axis=2
    ).astype(f16)

    shared = {
        "wq": np.ascontiguousarray(np.asarray(wq, f32)).astype(f16),
        "wk": np.ascontiguousarray(np.asarray(wk, f32)).astype(f16),
        "wv": np.ascontiguousarray(np.asarray(wv, f32)).astype(f16),
        "wo": np.ascontiguousarray(np.asarray(wo, f32)).astype(f16),
        "w1": w1_16,
        "w2": np.ascontiguousarray(np.asarray(w2, f32)).astype(f16),
        "w1b": np.ascontiguousarray(w1b),
        "lnd": lnd, "lnf": lnf, "encp": encp,
    }
    if _layers != L:
        for k in ("wq", "wk", "wv", "wo", "w1", "w2", "w1b", "lnd", "lnf"):
            shared[k] = np.ascontiguousarray(shared[k][:_layers])

    nc1 = _get_launch1(_layers)
    in_maps = [{"x0": x0s[c], **shared} for c in range(NCORES)]
    res1 = run_bass_kernel_spmd(nc1, in_maps, core_ids=list(range(NCORES)),
                                trace=_trace)
    hparts = [res1.results[c]["hpart"] for c in range(NCORES)]

    pooled = [np.sum([np.asarray(hparts[b * T + t], f32) for t in range(T)],
                     axis=0, dtype=f32) for b in range(B)]

    w_out_pad = np.zeros((D, VPAD), f16)
    w_out_pad[:, :V] = np.asarray(w_out, f32).astype(f16)

    nc2 = _get_launch2()
    in_maps2 = []
    for c in range(NCORES):
        b, s = divmod(c, 4)
        in_maps2.append({
            "pooled": pooled[b].astype(f16),
            "wsh": np.ascontiguousarray(w_out_pad[:, s * VSH:(s + 1) * VSH]),
        })
    res2 = run_bass_kernel_spmd(nc2, in_maps2, core_ids=list(range(NCORES)),
                                trace=_trace)

    out = np.empty((B, CTX, V), f32)
    for b in range(B):
        full = np.concatenate(
            [np.asarray(res2.results[b * 4 + s]["logits"], f32)
             for s in range(4)], axis=1)
        out[b] = full[:, :V]

    exec_ns = []
    for r in (res1, res2):
        if r.exec_time_ns is not None:
            exec_ns.append(r.exec_time_ns)
    if _trace and exec_ns:
        kernel.last_exec_ns = exec_ns
        kernel.last_results = (res1, res2)
    return out


# revision 19
# speedup vs baseline: 1.4950x; 1.0399x over previous
"""Trainium2 Bass kernel for MySpikeGPT (spiking linear-attention transformer).

Strategy: data-parallel over the 8 (T, B) slices across 8 NeuronCores.
Activations are feature-major [D, CTX] on-chip; layernorm statistics (over
features = partitions) are computed with an all-ones [128,128] stationary
matmul that replicates column sums onto all partitions. The spiking
attention (elementwise Q*K summed over sequence) is a fused
tensor_tensor_reduce on the vector engine.

v2 changes vs v1:
- fp16 weights + activations (fp32 PSUM accumulation): 2x DVE throughput,
  half the weight DMA, cheaper LDWEIGHTS. rel_err ~2.6e-3 (sim).
- LN scale via reciprocal_approx_fast (1 DVE op) instead of the 3.3us
  exact reciprocal; mean/var extraction moved to the scalar engine.
- relu(h) computed once per chunk and shared by the q/k/v and FFN GEMMs.
- V-apply folds the qk spike gate into the LN gamma/beta (qk >= 0 so
  relu(qk*(z+b)) == qk*relu(z+b)), decoupling the O-projection GEMM from
  the QK reduction.

Launch 1: embedding-encoder + 12 transformer layers -> relu(h)/T partials.
Host: sums the 4 per-timestep partials of each batch element.
Launch 2: pooled @ w_out, vocab-sharded 4-way per batch element.
"""

import sys

if "/opt/trn_rl_repo" not in sys.path:
    sys.path.insert(0, "/opt/trn_rl_repo")

import numpy as np

import concourse.bacc as bacc
import concourse.tile as tile
from concourse import mybir
from concourse.bass_utils import run_bass_kernel_spmd

P = 128
V, D, CTX, HH, HD, FF, L, T, B = 50257, 768, 1024, 12, 64, 3072, 12, 4, 2
KD = D // P          # 6 feature tiles of h/q/k/v
KF = FF // P         # 24 feature tiles of f1
EPS = 1e-5
LAMB = 10000.0
NCORES = 8
NCH = 2              # token chunks per CTX in launch 1
NT = CTX // NCH      # 512 tokens per chunk
FFQ = 4              # w1/w2 streamed in quarters of 768 columns/rows
VPAD = 51200         # vocab padded to 4 shards x 25 N-chunks x 512
VSH = VPAD // 4      # 12800 vocab columns per core

F32 = mybir.dt.float32
F16 = mybir.dt.float16
BF16 = mybir.dt.bfloat16     # squares: bf16 range dodges fp16 overflow
AF = mybir.ActivationFunctionType
ALU = mybir.AluOpType


def _poe() -> np.ndarray:
    i = np.arange(CTX, dtype=np.float32)[:, None]
    j = np.arange(D)
    expo = (j - (j % 2)).astype(np.float32) / D
    ang = i / (LAMB ** expo)
    return np.where(j % 2 == 0, np.sin(ang), np.cos(ang)).astype(np.float32)


def build_launch1(layers=L):
    nc = bacc.Bacc("TRN2", target_bir_lowering=False, debug=False,
                   num_devices=NCORES)

    x0_d = nc.dram_tensor("x0", [D, CTX], F16, kind="ExternalInput")
    wq_d = nc.dram_tensor("wq", [layers, D, D], F16, kind="ExternalInput")
    wk_d = nc.dram_tensor("wk", [layers, D, D], F16, kind="ExternalInput")
    wv_d = nc.dram_tensor("wv", [layers, D, D], F16, kind="ExternalInput")
    wo_d = nc.dram_tensor("wo", [layers, D, D], F16, kind="ExternalInput")
    w1_d = nc.dram_tensor("w1", [layers, D, FF], F16, kind="ExternalInput")
    w2_d = nc.dram_tensor("w2", [layers, FF, D], F16, kind="ExternalInput")
    # w1 row-sums, pre-replicated across 128 columns (sum-stat shortcut)
    w1b_d = nc.dram_tensor("w1b", [layers, D, P], F16, kind="ExternalInput")
    # per-layer D-sized LN params: qg qb kg kb vg vb og ob 2g 2b
    lnd_d = nc.dram_tensor("lnd", [layers, 10, D], F32, kind="ExternalInput")
    lnf_d = nc.dram_tensor("lnf", [layers, 2, FF], F32, kind="ExternalInput")
    enc_d = nc.dram_tensor("encp", [2, D], F32, kind="ExternalInput")
    hp_d = nc.dram_tensor("hpart", [D, CTX], F16, kind="ExternalOutput")

    with tile.TileContext(nc) as tc:
        with (
            tc.tile_pool(name="persist", bufs=1) as pers,
            tc.tile_pool(name="ybuf", bufs=3) as pyb,
            tc.tile_pool(name="y1buf", bufs=2) as py1,
            tc.tile_pool(name="sqbuf", bufs=2) as psq,
            tc.tile_pool(name="wpool", bufs=2) as pw,
            tc.tile_pool(name="w1bpool", bufs=2) as pwb,
            tc.tile_pool(name="stat16", bufs=4) as pst,
            tc.tile_pool(name="tmp32", bufs=4) as pt32,
            tc.tile_pool(name="ktmp", bufs=3) as pkt,
            tc.tile_pool(name="lnp", bufs=2) as plnp,
            tc.tile_pool(name="psmain", bufs=6, space="PSUM") as psp,
            tc.tile_pool(name="psstat", bufs=2, space="PSUM") as sps,
        ):
            ones32 = pers.tile([P, P], F32)
            nc.vector.memset(ones32, 1.0)
            ones = pers.tile([P, P], F16)
            nc.vector.tensor_copy(ones, ones32)
            ones_bf = pers.tile([P, P], BF16)
            nc.vector.tensor_copy(ones_bf, ones32)
            h = pers.tile([P, KD, CTX], F32)
            x0 = pers.tile([P, KD, CTX], F16)       # reused as hp at the end
            qa = pers.tile([P, KD, CTX], F16)       # Q, then att (o-proj rhs)
            th = pers.tile([P, NCH, KD, NT], F16)   # relu(h): qkv, then ffn
            qkraw = pers.tile([P, KD, NCH], F32)
            qk32 = pers.tile([P, KD], F32)
            qkr = pers.tile([P, KD], F32)
            gv = pers.tile([P, KD], F32)
            bpr = pers.tile([P, KD], F32)

            def stats(sum_ps, sq_ps, inv_n):
                """PSUM sum/sumsq -> (mean_f16, rsqrt(var)_f16), replicated."""
                m16 = pst.tile([P, NT], F16, tag="m16")
                nc.scalar.activation(m16, sum_ps[:], AF.Copy, scale=inv_n)
                msq = pt32.tile([P, NT], F32, tag="t32")
                nc.scalar.activation(msq, sum_ps[:], AF.Square, scale=inv_n)
                e2 = pt32.tile([P, NT], F32, tag="t32")
                nc.scalar.activation(e2, sq_ps[:], AF.Copy, scale=inv_n,
                                     bias=EPS)
                var = pt32.tile([P, NT], F32, tag="t32")
                nc.vector.tensor_sub(var, e2, msq)
                rinv = pt32.tile([P, NT], F32, tag="t32")
                nc.vector.reciprocal_approx_fast(out=rinv, in_=var)
                rs16 = pst.tile([P, NT], F16, tag="rs16")
                nc.scalar.activation(rs16, rinv, AF.Sqrt)
                return m16, rs16

            def copies_and_stats(pss, y, nm, inv_n, sq_extra=None):
                """PSUM tiles -> fp16 y[:, m]; emit sum+sq stat matmuls.
                Returns (m16, rs16). sq_extra: (sq_ps, start, stop) to
                accumulate sq into an existing PSUM instead of a new pair."""
                nmt = len(pss)
                sqt = psq.tile([P, nmt, NT], BF16, tag="sq")
                for m in range(nmt):
                    nc.scalar.activation(y[:, m], pss[m][:], AF.Copy)
                    nc.vector.tensor_mul(sqt[:, m], y[:, m], y[:, m])
                sum_ps = sps.tile([P, NT], F32, tag="sp")
                sq_ps = sps.tile([P, NT], F32, tag="sp")
                for m in range(nmt):
                    nc.tensor.matmul(sum_ps[:], ones[:], y[:, m],
                                     start=(m == 0), stop=(m == nmt - 1))
                for m in range(nmt):
                    nc.tensor.matmul(sq_ps[:], ones_bf[:], sqt[:, m],
                                     start=(m == 0), stop=(m == nmt - 1))
                return stats(sum_ps, sq_ps, inv_n)

            # ---------------- encoder layernorm ----------------
            encp_t = plnp.tile([P, 2, KD], F32, tag="lnd")
            nc.sync.dma_start(encp_t[:], enc_d.rearrange("r (k p) -> p r k", p=P))
            nc.sync.dma_start(x0[:], x0_d.rearrange("(k p) n -> p k n", p=P))
            for n in range(NCH):
                ns = slice(n * NT, (n + 1) * NT)
                sqt = psq.tile([P, KD, NT], BF16, tag="sq")
                sum_ps = sps.tile([P, NT], F32, tag="sp")
                sq_ps = sps.tile([P, NT], F32, tag="sp")
                for m in range(KD):
                    nc.vector.tensor_mul(sqt[:, m], x0[:, m, ns],
                                         x0[:, m, ns])
                for m in range(KD):
                    nc.tensor.matmul(sum_ps[:], ones[:], x0[:, m, ns],
                                     start=(m == 0), stop=(m == KD - 1))
                for m in range(KD):
                    nc.tensor.matmul(sq_ps[:], ones_bf[:], sqt[:, m],
                                     start=(m == 0), stop=(m == KD - 1))
                m16, rs16 = stats(sum_ps, sq_ps, 1.0 / D)
                for m in range(KD):
                    nc.vector.tensor_sub(x0[:, m, ns], x0[:, m, ns], m16)
                    nc.vector.tensor_mul(x0[:, m, ns], x0[:, m, ns], rs16)
                    nc.scalar.activation(h[:, m, ns], x0[:, m, ns],
                                         AF.Identity,
                                         scale=encp_t[:, 0, m:m + 1],
                                         bias=encp_t[:, 1, m:m + 1])

            # ---------------- transformer layers ----------------
            for l in range(layers):
                lnd_t = plnp.tile([P, 10, KD], F32, tag="lnd")
                nc.sync.dma_start(lnd_t[:],
                                  lnd_d[l].rearrange("r (k p) -> p r k", p=P))
                lnf_t = plnp.tile([P, 2, KF], F32, tag="lnf")
                nc.sync.dma_start(lnf_t[:],
                                  lnf_d[l].rearrange("r (k p) -> p r k", p=P))

                # shared relu(h) for q/k/v
                for n in range(NCH):
                    ns = slice(n * NT, (n + 1) * NT)
                    for m in range(KD):
                        nc.scalar.activation(th[:, n, m], h[:, m, ns], AF.Relu)

                # --- q, k, v, o projections ---
                for mode, w_src, grow, brow in (
                    ("q", wq_d, 0, 1), ("k", wk_d, 2, 3),
                    ("v", wv_d, 4, 5), ("o", wo_d, 6, 7),
                ):
                    w_t = pw.tile([P, KD, D], F16, tag="w")
                    nc.sync.dma_start(
                        w_t[:], w_src[l].rearrange("(k p) m -> p k m", p=P))
                    for n in range(NCH):
                        ns = slice(n * NT, (n + 1) * NT)
                        if mode == "o":
                            rhs = [qa[:, k, ns] for k in range(KD)]
                        else:
                            rhs = [th[:, n, k] for k in range(KD)]
                        pss = []
                        for m in range(KD):
                            ps = psp.tile([P, NT], F32, tag="yp")
                            pss.append(ps)
                            for k in range(KD):
                                nc.tensor.matmul(
                                    ps[:], w_t[:, k, m * P:(m + 1) * P],
                                    rhs[k],
                                    start=(k == 0), stop=(k == KD - 1))
                        y = pyb.tile([P, KD, NT], F16, tag="y")
                        m16, rs16 = copies_and_stats(pss, y, NT, 1.0 / D)
                        for m in range(KD):
                            nc.vector.tensor_sub(y[:, m], y[:, m], m16)
                            if mode == "o":
                                nc.vector.scalar_tensor_tensor(
                                    y[:, m], in0=y[:, m],
                                    scalar=lnd_t[:, grow, m:m + 1], in1=rs16,
                                    op0=ALU.mult, op1=ALU.mult)
                                # h += y + ob
                                nc.vector.scalar_tensor_tensor(
                                    h[:, m, ns], in0=y[:, m],
                                    scalar=lnd_t[:, brow, m:m + 1],
                                    in1=h[:, m, ns],
                                    op0=ALU.add, op1=ALU.add)
                            else:
                                nc.vector.tensor_mul(y[:, m], y[:, m], rs16)
                                if mode == "v":
                                    # relu(qk*(z*g+b)) = qk*relu(z*g+b)
                                    nc.scalar.activation(
                                        qa[:, m, ns], y[:, m], AF.Relu,
                                        scale=gv[:, m:m + 1],
                                        bias=bpr[:, m:m + 1])
                                elif mode == "q":
                                    nc.scalar.activation(
                                        qa[:, m, ns], y[:, m], AF.Relu,
                                        scale=lnd_t[:, grow, m:m + 1],
                                        bias=lnd_t[:, brow, m:m + 1])
                                else:  # k: relu, then fused QK reduce
                                    kt = pkt.tile([P, NT], F16, tag="kt")
                                    nc.scalar.activation(
                                        kt, y[:, m], AF.Relu,
                                        scale=lnd_t[:, grow, m:m + 1],
                                        bias=lnd_t[:, brow, m:m + 1])
                                    prod = pkt.tile([P, NT], F16, tag="prod")
                                    nc.vector.tensor_mul(prod, qa[:, m, ns],
                                                         kt)
                                    nc.vector.tensor_reduce(
                                        qkraw[:, m, n:n + 1], prod,
                                        axis=mybir.AxisListType.X, op=ALU.add)
                    if mode == "k":
                        # qk = relu(sum_seq q*k); fold into v's LN params
                        nc.vector.tensor_add(qk32[:, :], qkraw[:, :, 0],
                                             qkraw[:, :, 1])
                        nc.scalar.activation(qkr[:, :], qk32[:, :], AF.Relu)
                        nc.vector.tensor_mul(gv[:, :], lnd_t[:, 4, :],
                                             qkr[:, :])
                        nc.vector.tensor_mul(bpr[:, :], lnd_t[:, 5, :],
                                             qkr[:, :])

                # --- FFN ---
                w1b_t = pwb.tile([P, KD, P], F16, tag="w1b")
                nc.sync.dma_start(
                    w1b_t[:], w1b_d[l].rearrange("(k p) c -> p k c", p=P))
                # relu(h) after attention update (reuses th buffer)
                for n in range(NCH):
                    ns = slice(n * NT, (n + 1) * NT)
                    for m in range(KD):
                        nc.scalar.activation(th[:, n, m], h[:, m, ns], AF.Relu)
                y1s = []
                for n in range(NCH):
                    sum_ps = sps.tile([P, NT], F32, tag="sp")
                    sq_ps = sps.tile([P, NT], F32, tag="sp")
                    # sum-stat via w1 row-sums
                    for k in range(KD):
                        nc.tensor.matmul(sum_ps[:], w1b_t[:, k], th[:, n, k],
                                         start=(k == 0), stop=(k == KD - 1))
                    y1 = py1.tile([P, KF, NT], F16, tag="y1")
                    y1s.append(y1)
                    for fq in range(FFQ):
                        w_t = pw.tile([P, KD, D], F16, tag="w")
                        nc.sync.dma_start(
                            w_t[:],
                            w1_d[l][:, fq * D:(fq + 1) * D].rearrange(
                                "(k p) m -> p k m", p=P))
                        pss = []
                        for m in range(KD):
                            ps = psp.tile([P, NT], F32, tag="yp")
                            pss.append(ps)
                            for k in range(KD):
                                nc.tensor.matmul(
                                    ps[:], w_t[:, k, m * P:(m + 1) * P],
                                    th[:, n, k],
                                    start=(k == 0), stop=(k == KD - 1))
                        sqt = psq.tile([P, KD, NT], BF16, tag="sq")
                        for m in range(KD):
                            mg = fq * KD + m
                            nc.scalar.activation(y1[:, mg], pss[m][:], AF.Copy)
                            nc.vector.tensor_mul(sqt[:, m], y1[:, mg],
                                                 y1[:, mg])
                        for m in range(KD):
                            mg = fq * KD + m
                            nc.tensor.matmul(sq_ps[:], ones_bf[:], sqt[:, m],
                                             start=(mg == 0),
                                             stop=(mg == KF - 1))
                    m16, rs16 = stats(sum_ps, sq_ps, 1.0 / FF)
                    for mg in range(KF):
                        nc.vector.tensor_sub(y1[:, mg], y1[:, mg], m16)
                        nc.vector.tensor_mul(y1[:, mg], y1[:, mg], rs16)
                        nc.scalar.activation(y1[:, mg], y1[:, mg], AF.Relu,
                                             scale=lnf_t[:, 0, mg:mg + 1],
                                             bias=lnf_t[:, 1, mg:mg + 1])
                # f2 = f1 @ w2, contracting all 24 feature tiles
                for n in range(NCH):
                    ns = slice(n * NT, (n + 1) * NT)
                    y1 = y1s[n]
                    pss2 = [psp.tile([P, NT], F32, tag="yp", name=f"ps2_{m}")
                            for m in range(KD)]
                    for qq in range(FFQ):
                        w_t = pw.tile([P, KD, D], F16, tag="w")
                        nc.sync.dma_start(
                            w_t[:],
                            w2_d[l][qq * D:(qq + 1) * D].rearrange(
                                "(k p) m -> p k m", p=P))
                        for kk in range(KD):
                            for m in range(KD):
                                nc.tensor.matmul(
                                    pss2[m][:], w_t[:, kk, m * P:(m + 1) * P],
                                    y1[:, qq * KD + kk],
                                    start=(qq == 0 and kk == 0),
                                    stop=(qq == FFQ - 1 and kk == KD - 1))
                    yf = pyb.tile([P, KD, NT], F16, tag="y")
                    m16, rs16 = copies_and_stats(pss2, yf, NT, 1.0 / D)
                    for m in range(KD):
                        nc.vector.tensor_sub(yf[:, m], yf[:, m], m16)
                        nc.vector.scalar_tensor_tensor(
                            yf[:, m], in0=yf[:, m],
                            scalar=lnd_t[:, 8, m:m + 1], in1=rs16,
                            op0=ALU.mult, op1=ALU.mult)
                        nc.vector.scalar_tensor_tensor(
                            h[:, m, ns], in0=yf[:, m],
                            scalar=lnd_t[:, 9, m:m + 1], in1=h[:, m, ns],
                            op0=ALU.add, op1=ALU.add)

            # ---------------- pooled partial ----------------
            for m in range(KD):
                nc.scalar.activation(x0[:, m], h[:, m], AF.Relu,
                                     scale=1.0 / T)
            nc.sync.dma_start(hp_d.rearrange("(k p) n -> p k n", p=P), x0[:])

    nc.compile()
    return nc


def build_launch2():
    nc = bacc.Bacc("TRN2", target_bir_lowering=False, debug=False,
                   num_devices=NCORES)
    pooled_d = nc.dram_tensor("pooled", [D, CTX], F16, kind="ExternalInput")
    wsh_d = nc.dram_tensor("wsh", [D, VSH], F16, kind="ExternalInput")
    out_d = nc.dram_tensor("logits", [CTX, VSH], F16, kind="ExternalOutput")

    NV = VSH // 512  # 25 vocab chunks per core

    with tile.TileContext(nc) as tc:
        with (
            tc.tile_pool(name="pld", bufs=1) as pld,
            tc.tile_pool(name="wp", bufs=3) as pw,
            tc.tile_pool(name="op", bufs=4) as po,
            tc.tile_pool(name="ps", bufs=8, space="PSUM") as psp,
        ):
            pt = pld.tile([P, KD, CTX], F16)
            nc.sync.dma_start(pt[:], pooled_d.rearrange("(k p) n -> p k n",
                                                        p=P))
            for v in range(NV):
                w_t = pw.tile([P, KD, 512], F16, tag="w")
                nc.sync.dma_start(
                    w_t[:],
                    wsh_d[:, v * 512:(v + 1) * 512].rearrange(
                        "(k p) n -> p k n", p=P))
                for m in range(CTX // P):
                    ps = psp.tile([P, 512], F32, tag="ps")
                    for k in range(KD):
                        nc.tensor.matmul(ps[:], pt[:, k, m * P:(m + 1) * P],
                                         w_t[:, k],
                                         start=(k == 0), stop=(k == KD - 1))
                    ot = po.tile([P, 512], F16, tag="o")
                    nc.scalar.activation(ot, ps[:], AF.Copy)
                    nc.sync.dma_start(
                        out_d[m * P:(m + 1) * P, v * 512:(v + 1) * 512], ot)
    nc.compile()
    return nc


_CACHE = {}


def _get_launch1(layers=L):
    key = ("l1", layers)
    if key not in _CACHE:
        _CACHE[key] = build_launch1(layers)
    return _CACHE[key]


def _get_launch2():
    if "l2" not in _CACHE:
        _CACHE["l2"] = build_launch2()
    return _CACHE["l2"]


def kernel(tokens, emb, enc_g, enc_b, wq, wk, wv, wo,
           lnq_g, lnq_b, lnk_g, lnk_b, lnv_g, lnv_b, lno_g, lno_b,
           w1, ln1_g, ln1_b, w2, ln2_g, ln2_b, w_out,
           _layers=L, _trace=False):
    f32, f16 = np.float32, np.float16
    tokens = np.asarray(tokens)
    poe = _poe()

    # per-core (t, b) slices of the encoded input, feature-major
    x0s = []
    for c in range(NCORES):
        b, t = divmod(c, T)
        x = np.asarray(emb, f32)[tokens[t, b]] + poe       # [CTX, D]
        x0s.append(np.ascontiguousarray(x.T).astype(f16))  # [D, CTX]

    lnd = np.ascontiguousarray(
        np.stack([lnq_g, lnq_b, lnk_g, lnk_b, lnv_g, lnv_b,
                  lno_g, lno_b, ln2_g, ln2_b], axis=1), f32)  # [L,10,D]
    lnf = np.ascontiguousarray(
        np.stack([ln1_g, ln1_b], axis=1), f32)                # [L,2,FF]
    encp = np.stack([enc_g, enc_b]).astype(f32)               # [2,D]

    w1_16 = np.ascontiguousarray(np.asarray(w1, f32)).astype(f16)
    w1b = np.repeat(
        w1_16.astype(f32).sum(axis=2, dtype=f32)[:, :, None], P, axis=2
    ).astype(f16)

    shared = {
        "wq": np.ascontiguousarray(np.asarray(wq, f32)).astype(f16),
        "wk": np.ascontiguousarray(np.asarray(wk, f32)).astype(f16),
        "wv": np.ascontiguousarray(np.asarray(wv, f32)).astype(f16),
        "wo": np.ascontiguousarray(np.asarray(wo, f32)).astype(f16),
        "w1": w1_16,
        "w2": np.ascontiguousarray(np.asarray(w2, f32)).astype(f16),
        "w1b": np.ascontiguousarray(w1b),
        "lnd": lnd, "lnf": lnf, "encp": encp,
    }
    if _layers != L:
        for k in ("wq", "wk", "wv", "wo", "w1", "w2", "w1b", "lnd", "lnf"):
            shared[k] = np.ascontiguousarray(shared[k][:_layers])

    nc1 = _get_launch1(_layers)
    in_maps = [{"x0": x0s[c], **shared} for c in range(NCORES)]
    res1 = run_bass_kernel_spmd(nc1, in_maps, core_ids=list(range(NCORES)),
                                trace=_trace)
    hparts = [res1.results[c]["hpart"] for c in range(NCORES)]

    pooled = [np.sum([np.asarray(hparts[b * T + t], f32) for t in range(T)],
                     axis=0, dtype=f32) for b in range(B)]

    w_out_pad = np.zeros((D, VPAD), f16)
    w_out_pad[:, :V] = np.asarray(w_out, f32).astype(f16)

    nc2 = _get_launch2()
    in_maps2 = []
    for c in range(NCORES):
        b, s = divmod(c, 4)
        in_maps2.append({
            "pooled": pooled[b].astype(f16),
            "wsh": np.ascontiguousarray(w_out_pad[:, s * VSH:(s + 1) * VSH]),
        })
    res2 = run_bass_kernel_spmd(nc2, in_maps2, core_ids=list(range(NCORES)),
                                trace=_trace)

    out = np.empty((B, CTX, V), f32)
    for b in range(B):
        full = np.concatenate(
            [np.asarray(res2.results[b * 4 + s]["logits"], f32)
             for s in range(4)], axis=1)
        out[b] = full[:, :V]

    exec_ns = []
    for r in (res1, res2):
        if r.exec_time_ns is not None:
            exec_ns.append(r.exec_time_ns)
    if _trace and exec_ns:
        kernel.last_exec_ns = exec_ns
        kernel.last_results = (res1, res2)
    return out
